# revision 21
# baseline (speedup 1.0000x reference)
"""CapsuleLayer (dynamic routing) Trainium2 Bass kernel.

Full-input contract: kernel(inputs, W) -> [256, 10, 16, 1] f32.
Data-parallel over batch: 8 cores x 32 batches, W replicated.

Math restructuring vs the reference:
  - routing logits are always b_t = u_hat * V_t with V_t = sum of previous
    squashed outputs (broadcast over IC), so no [B,NC,IC,DC] logits tensor is
    ever materialized; only the running V[b,n,d] is kept.
  - pass 1 (uniform softmax) reduces to s1 = 0.1 * sum_i u_hat, computed as a
    masked matmul fold on the PE during u_hat production.

Everything is kept in f32: the routing iteration chaotically amplifies
perturbations (~700x), so bf16/f16 intermediates destroy accuracy. The only
reduced-precision concession is float32r on the capsule-fold matmuls (N=480
keeps them at full PE rate vs 4 cyc/row for plain fp32).

Per core the 32 local batches are processed as 2 serial sub-batches of 16 so
the f32 u_hat stays SBUF-resident (no DRAM spill):
  partition p = i8*16 + b   (8 input capsules packed per "group", 144 groups)
  u_hat: 12 chunk tiles [128, 12*160] in a 12-slot pool; sub-batch 2's
    production reuses sub-batch 1's slots chunk-by-chunk, so its DMA/PE work
    overlaps sub-batch 1's final routing pass.
  production: u[p, g*160+nd] = sum_{r=(i8,k)} LT[r, g*128+p] * WR[r, g*160+nd]
    with LT the block-diagonal input transposes (rows r = i8*8+k, 16-col
    diagonal blocks) and WR the matching W slices - both prepacked on host.
  capsule fold: s[b, nd] = sum_p mask[p, b] * w[p, nd] via PE (mask = eye(16)
    tiled 8x), accumulated 3 groups wide ([16, 480] PSUM) to amortize weight
    loads; the 3 column blocks are summed on DVE afterwards.
Engine split per routing pass chunk:
  DVE: logits-mult, y = e*u_hat, reciprocal; ScalarE: exp;
  GpSimd: softmax denominator pair-tree + normalize-mult; PE: capsule folds.
"""

import os
import sys

import numpy as np

sys.path.insert(0, "/opt/trn_rl_repo")

B, IC, ID = 256, 1152, 8
NC, DC = 10, 16
NCORES = 8
BC = B // NCORES            # 32 batches per core
SB = 2                      # sub-batches per core
BB = BC // SB               # 16 batches per sub-batch
IPK = 8                     # input capsules packed per group
G2 = IC // IPK              # 144 groups
K2 = IPK * ID               # 64 contraction rows
ND = NC * DC                # 160
FREE2 = G2 * ND             # 23040
CH = 12                     # chunk size in groups (production & routing)
NCH = G2 // CH              # 12 chunks
PSUM_GRP = 3                # groups per PSUM bank tile (3*160*4B < 2KB)
EPS = 1e-7

_CACHE = {}


def _build_nc():
    import concourse.bacc as bacc
    import concourse.mybir as mybir
    import concourse.tile as tile

    F32 = mybir.dt.float32
    F32R = mybir.dt.float32r
    ALU = mybir.AluOpType
    ACTF = mybir.ActivationFunctionType

    nc = bacc.Bacc()
    # lt: sub-batch 2's block-diagonal stationary tiles (DMA'd under the
    # routing overlap). Sub-batch 1 ships compact (xt) and is expanded
    # on-device (GpSimd mask-multiply) to cut the serial P1 DMA.
    lt_d = nc.dram_tensor("lt", [K2, G2 * 128], F32, kind="ExternalInput")
    xt_d = nc.dram_tensor("xt", [K2, G2 * BB], F32, kind="ExternalInput")
    mlt_d = nc.dram_tensor("mlt", [K2, 128], F32, kind="ExternalInput")
    wr_d = nc.dram_tensor("wr", [K2, FREE2], F32, kind="ExternalInput")
    mask_d = nc.dram_tensor("mask", [128, BB], F32, kind="ExternalInput")
    out_d = nc.dram_tensor("out", [BC, ND], F32, kind="ExternalOutput")

    with tile.TileContext(nc) as tc:
        with (
            tc.tile_pool(name="const", bufs=1) as cpool,
            tc.tile_pool(name="sq", bufs=1) as qpool,
            tc.tile_pool(name="uhp", bufs=NCH) as uhp,
            tc.tile_pool(name="psw", bufs=2, space="PSUM") as swpool,
            tc.tile_pool(name="ltp", bufs=3) as ltp,
            tc.tile_pool(name="wrp", bufs=3) as wrp,
            tc.tile_pool(name="pprod", bufs=6, space="PSUM") as pprod,
            tc.tile_pool(name="x", bufs=3) as xpool,
            tc.tile_pool(name="y", bufs=3) as ypool,
            tc.tile_pool(name="dn", bufs=2) as dnpool,
            tc.tile_pool(name="s1t", bufs=1) as s1pool,
            tc.tile_pool(name="rv", bufs=2) as rvpool,
        ):
            mask_t = cpool.tile([128, BB], F32)
            nc.sync.dma_start(mask_t[:], mask_d[:])
            mlt_t = cpool.tile([K2, 128], F32)
            nc.sync.dma_start(mlt_t[:], mlt_d[:])
            mask_r = cpool.tile([128, BB], F32R)
            nc.vector.tensor_copy(mask_r[:], mask_t[:])
            # V is kept replicated 8x across partitions (p%16 = b) so the
            # squash chain directly produces the broadcast tile for the next
            # pass's logits; only one small broadcast DMA per transition.
            V = cpool.tile([128, ND], F32)

            def collapse3(ps_w, sc):
                # s_sb = (blk0+blk1+blk2) of [16, 480] PSUM -> [16, 160] SBUF
                cw = qpool.tile([BB, PSUM_GRP * ND], F32, tag="c3_w")
                nc.scalar.activation(cw[:], ps_w[:], ACTF.Copy, scale=sc)
                s3 = qpool.tile([BB, ND], F32, tag="c3_a")
                nc.vector.tensor_add(
                    s3[:], cw[:, 0:ND], cw[:, ND:2 * ND]
                )
                s = qpool.tile([BB, ND], F32, tag="c3_s")
                nc.vector.tensor_add(s[:], s3[:], cw[:, 2 * ND:3 * ND])
                return s

            def bcast16(s):
                # replicate [16, ND] -> [128, ND] (8 partition quadrants)
                s128 = qpool.tile([128, ND], F32, tag="s128")
                for q in range(IPK):
                    nc.sync.dma_start(s128[q * BB:(q + 1) * BB, :], s[:, :])
                return s128

            def squash(s, vt, P=128):
                # vt = squash(s); [P, ND] f32, tiny
                sq = qpool.tile([P, ND], F32, tag="sq_sq")
                nc.vector.tensor_mul(sq[:], s[:], s[:])
                se = qpool.tile([P, ND], F32, tag="sq_se")
                nc.vector.tensor_scalar_add(se[:], sq[:], EPS)
                a = qpool.tile([P, ND], F32, tag="sq_a")
                nc.scalar.activation(a[:], se[:], ACTF.Sqrt)
                d2 = qpool.tile([P, ND], F32, tag="sq_d2")
                nc.vector.scalar_tensor_tensor(
                    d2[:], sq[:], 1.0, a[:], op0=ALU.add, op1=ALU.mult
                )
                r = qpool.tile([P, ND], F32, tag="sq_r")
                nc.vector.reciprocal(r[:], d2[:])
                t1 = qpool.tile([P, ND], F32, tag="sq_t1")
                nc.vector.tensor_mul(t1[:], s[:], sq[:])
                nc.vector.tensor_mul(vt[:], t1[:], r[:])

            for s_i in range(SB):
                # ---------- production: u_hat + s1 fold ----------
                uch = []
                s1parts = []
                for c in range(NCH):
                    g0 = c * CH
                    ltt = ltp.tile([K2, CH * 128], F32, tag="ltt")
                    if s_i == 0:
                        xtt = ltp.tile([K2, CH * BB], F32, tag="xtt")
                        nc.sync.dma_start(
                            xtt[:], xt_d[:, g0 * BB:(g0 + CH) * BB]
                        )
                        ltt4 = ltt[:].rearrange(
                            "p (g i b) -> p g i b", i=IPK, b=BB
                        )
                        xt_b = (
                            xtt[:]
                            .rearrange("p (g b) -> p g b", b=BB)
                            .unsqueeze(2)
                            .broadcast_to([K2, CH, IPK, BB])
                        )
                        ml_b = (
                            mlt_t[:]
                            .rearrange("p (i b) -> p i b", b=BB)
                            .unsqueeze(1)
                            .broadcast_to([K2, CH, IPK, BB])
                        )
                        nc.gpsimd.tensor_tensor(ltt4, xt_b, ml_b, ALU.mult)
                    else:
                        nc.sync.dma_start(
                            ltt[:],
                            lt_d[:, g0 * 128:(g0 + CH) * 128],
                        )
                    wrt = wrp.tile([K2, CH * ND], F32)
                    nc.sync.dma_start(
                        wrt[:], wr_d[:, g0 * ND:(g0 + CH) * ND]
                    )
                    u = uhp.tile([128, CH * ND], F32, tag="uh")
                    uch.append(u)
                    for t3 in range(CH // PSUM_GRP):
                        pt = pprod.tile([128, PSUM_GRP * ND], F32)
                        for j in range(PSUM_GRP):
                            gl = t3 * PSUM_GRP + j
                            nc.tensor.matmul(
                                pt[:, j * ND:(j + 1) * ND],
                                ltt[:, gl * 128:(gl + 1) * 128],
                                wrt[:, gl * ND:(gl + 1) * ND],
                                start=True,
                                stop=True,
                            )
                        lo = t3 * PSUM_GRP * ND
                        hi = (t3 + 1) * PSUM_GRP * ND
                        nc.scalar.copy(u[:, lo:hi], pt[:])
                    # s1 partial: sum over the chunk's 12 groups on DVE
                    u3c = u[:].rearrange("p (g nd) -> p g nd", nd=ND)
                    t6 = s1pool.tile([128, 6 * ND], F32, tag="s1a")
                    t63 = t6[:].rearrange("p (g nd) -> p g nd", nd=ND)
                    nc.vector.tensor_tensor(
                        t63, u3c[:, 0:12:2, :], u3c[:, 1:12:2, :], ALU.add
                    )
                    t3_ = s1pool.tile([128, 3 * ND], F32, tag="s1b")
                    t33 = t3_[:].rearrange("p (g nd) -> p g nd", nd=ND)
                    nc.vector.tensor_tensor(
                        t33, t63[:, 0:6:2, :], t63[:, 1:6:2, :], ALU.add
                    )
                    sp = s1pool.tile([128, ND], F32, tag="s1p")
                    nc.vector.tensor_add(sp[:], t33[:, 0, :], t33[:, 1, :])
                    nc.vector.tensor_add(sp[:], sp[:], t33[:, 2, :])
                    if c == 0:
                        s1acc = cpool.tile([128, ND], F32, tag="s1acc")
                        nc.vector.tensor_copy(s1acc[:], sp[:])
                    else:
                        nc.vector.tensor_add(s1acc[:], s1acc[:], sp[:])
                # partition fold (i8 quadrants -> b) via one plain-f32 matmul
                ps1 = swpool.tile([BB, ND], F32, tag="psw")
                nc.tensor.matmul(ps1[:], mask_t[:], s1acc[:],
                                 start=True, stop=True)
                s1 = qpool.tile([BB, ND], F32, tag="c3_s")
                nc.scalar.activation(s1[:], ps1[:], ACTF.Copy, scale=0.1)
                squash(bcast16(s1), V)

                # ---------- routing passes 2 and 3 ----------
                for t in (2, 3):
                    ps_w = swpool.tile([BB, PSUM_GRP * ND], F32, tag="psw")
                    n_fold = 0
                    for c in range(NCH):
                        u = uch[c]
                        x = xpool.tile([128, CH * ND], F32)
                        x3 = x[:].rearrange("p (g nd) -> p g nd", nd=ND)
                        u3 = u[:].rearrange("p (g nd) -> p g nd", nd=ND)
                        vb_b = V[:].unsqueeze(1).broadcast_to(
                            [128, CH, ND]
                        )
                        # logits = u_hat * V (broadcast over groups)
                        nc.vector.tensor_tensor(x3, u3, vb_b, ALU.mult)
                        nc.scalar.activation(x[:], x[:], ACTF.Exp)
                        # y = e * u_hat, parallel with the GP denom tree;
                        # alternate DVE/GP per chunk for engine balance.
                        # pass 2 folds in exact f32 (its error feeds back
                        # through V and is chaotically amplified); pass 3
                        # folds in f32r (error hits the output directly).
                        y = ypool.tile([128, CH * ND],
                                       F32 if t == 2 else F32R,
                                       tag="y")
                        y_eng = nc.vector if c % 2 == 0 else nc.gpsimd
                        y_eng.tensor_tensor(
                            y[:], x[:], u[:], ALU.mult
                        )
                        # denominator pair-tree over n on GpSimd
                        x4 = x[:].rearrange(
                            "p (g n d) -> p g n d", n=NC, d=DC
                        )
                        d5 = dnpool.tile([128, CH * 5 * DC], F32, tag="d5")
                        d53 = d5[:].rearrange(
                            "p (g n d) -> p g n d", n=5, d=DC
                        )
                        nc.gpsimd.tensor_tensor(
                            d53, x4[:, :, 0:10:2, :], x4[:, :, 1:10:2, :],
                            ALU.add,
                        )
                        dp = dnpool.tile([128, CH * 2 * DC], F32, tag="dp")
                        dp3 = dp[:].rearrange(
                            "p (g n d) -> p g n d", n=2, d=DC
                        )
                        nc.gpsimd.tensor_tensor(
                            dp3, d53[:, :, 0:4:2, :], d53[:, :, 1:4:2, :],
                            ALU.add,
                        )
                        dn = dnpool.tile([128, CH * DC], F32, tag="dn")
                        dn3 = dn[:].rearrange("p (g d) -> p g d", d=DC)
                        nc.gpsimd.tensor_tensor(
                            dn3, dp3[:, :, 0, :], dp3[:, :, 1, :], ALU.add
                        )
                        nc.gpsimd.tensor_tensor(
                            dn3, dn3, d53[:, :, 4, :], ALU.add
                        )
                        rv = rvpool.tile([128, CH * DC], F32)
                        nc.vector.reciprocal(rv[:], dn[:])
                        rv_b = (
                            rv[:]
                            .rearrange("p (g d) -> p g d", d=DC)
                            .unsqueeze(2)
                            .broadcast_to([128, CH, NC, DC])
                        )
                        y4 = y[:].rearrange(
                            "p (g n d) -> p g n d", n=NC, d=DC
                        )
                        nc.gpsimd.tensor_tensor(y4, y4, rv_b, ALU.mult)
                        mk = mask_t if t == 2 else mask_r
                        for j3 in range(CH // PSUM_GRP):
                            nc.tensor.matmul(
                                ps_w[:],
                                mk[:],
                                y[:, j3 * PSUM_GRP * ND:
                                  (j3 + 1) * PSUM_GRP * ND],
                                start=(n_fold == 0),
                                stop=(n_fold == G2 // PSUM_GRP - 1),
                            )
                            n_fold += 1
                    s_t = collapse3(ps_w, 1.0)
                    vt = qpool.tile([128, ND], F32, tag="vt")
                    squash(bcast16(s_t), vt)
                    if t == 2:
                        nc.vector.tensor_add(V[:], V[:], vt[:])
                    else:
                        nc.sync.dma_start(
                            out_d[s_i * BB:(s_i + 1) * BB, :],
                            vt[0:BB, :],
                        )
    nc.finalize()
    return nc


def _host_pack(inputs, W):
    """Build per-core LT, shared WR and mask, all f32."""
    inputs = np.ascontiguousarray(inputs, dtype=np.float32)
    W = np.ascontiguousarray(W, dtype=np.float32)

    # WR[r=(i8*8+k), g*160 + n*16 + d] = W[n, g*8+i8, d, k]
    W6 = W.reshape(NC, G2, IPK, DC, ID)
    wr = np.ascontiguousarray(
        W6.transpose(2, 4, 1, 0, 3).reshape(K2, FREE2)
    )

    mask = np.ascontiguousarray(
        np.tile(np.eye(BB, dtype=np.float32), (IPK, 1))
    )
    # mlt[r=(i8*8+k), i8'*16+b] = (i8 == i8')
    mlt = np.zeros((K2, 128), dtype=np.float32)
    for i8 in range(IPK):
        mlt[i8 * ID:(i8 + 1) * ID, i8 * BB:(i8 + 1) * BB] = 1.0

    lts, xts = [], []
    for core in range(NCORES):
        xc = inputs[core * BC:(core + 1) * BC]              # [BC, IC, ID]
        x6 = xc.reshape(SB, BB, G2, IPK, ID)                # [s, b, g, i8, k]
        # sub-batch 2: full block-diagonal layout
        lt = np.zeros((K2, G2, 128), dtype=np.float32)
        for i8 in range(IPK):
            lt[i8 * ID:(i8 + 1) * ID, :, i8 * BB:(i8 + 1) * BB] = (
                x6[1, :, :, i8, :].transpose(2, 1, 0)       # [k, g, b]
            )
        lts.append(np.ascontiguousarray(lt.reshape(K2, G2 * 128)))
        # sub-batch 1: compact transposed inputs
        xt = np.zeros((K2, G2, BB), dtype=np.float32)
        for i8 in range(IPK):
            xt[i8 * ID:(i8 + 1) * ID] = x6[0, :, :, i8, :].transpose(2, 1, 0)
        xts.append(np.ascontiguousarray(xt.reshape(K2, G2 * BB)))
    return lts, xts, wr, mask, mlt


def kernel(inputs, W):
    from concourse.bass_utils import run_bass_kernel_spmd

    if "nc" not in _CACHE:
        _CACHE["nc"] = _build_nc()
    nc = _CACHE["nc"]

    lts, xts, wr, mask, mlt = _host_pack(np.asarray(inputs), np.asarray(W))
    in_maps = [
        {"lt": lts[c], "xt": xts[c], "wr": wr, "mask": mask, "mlt": mlt}
        for c in range(NCORES)
    ]
    res = run_bass_kernel_spmd(nc, in_maps, core_ids=list(range(NCORES)))
    outs = [
        np.asarray(res.results[c]["out"]).reshape(BC, NC, DC, 1)
        for c in range(NCORES)
    ]
    return np.concatenate(outs, axis=0).astype(np.float32)


if __name__ == "__main__":
    rng = np.random.default_rng(0)
    x = rng.standard_normal((B, IC, ID), dtype=np.float32)
    w = rng.standard_normal((NC, IC, DC, ID), dtype=np.float32) * 0.1
    out = kernel(x, w)
    print(out.shape, out.dtype)


# revision 22
# speedup vs baseline: 8.1854x; 8.1854x over previous
"""CapsuleLayer (dynamic routing) Trainium2 Bass kernel.

Full-input contract: kernel(inputs, W) -> [256, 10, 16, 1] f32.
Data-parallel over batch: 8 cores x 32 batches, W replicated.

Math restructuring vs the reference:
  - routing logits are always b_t = u_hat * V_t with V_t = sum of previous
    squashed outputs (broadcast over IC), so no [B,NC,IC,DC] logits tensor is
    ever materialized; only the running V[b,n,d] is kept.
  - pass 1 (uniform softmax) reduces to s1 = 0.1 * sum_i u_hat, computed as a
    masked matmul fold on the PE during u_hat production.

Everything is kept in f32: the routing iteration chaotically amplifies
perturbations (~700x), so bf16/f16 intermediates destroy accuracy. The only
reduced-precision concession is float32r on the capsule-fold matmuls (N=480
keeps them at full PE rate vs 4 cyc/row for plain fp32).

Per core the 32 local batches are processed as 2 serial sub-batches of 16 so
the f32 u_hat stays SBUF-resident (no DRAM spill):
  partition p = i8*16 + b   (8 input capsules packed per "group", 144 groups)
  u_hat: 12 chunk tiles [128, 12*160] in a 12-slot pool; sub-batch 2's
    production reuses sub-batch 1's slots chunk-by-chunk, so its DMA/PE work
    overlaps sub-batch 1's final routing pass.
  production: u[p, g*160+nd] = sum_{r=(i8,k)} LT[r, g*128+p] * WR[r, g*160+nd]
    with LT the block-diagonal input transposes (rows r = i8*8+k, 16-col
    diagonal blocks) and WR the matching W slices - both prepacked on host.
  capsule fold: s[b, nd] = sum_p mask[p, b] * w[p, nd] via PE (mask = eye(16)
    tiled 8x), accumulated 3 groups wide ([16, 480] PSUM) to amortize weight
    loads; the 3 column blocks are summed on DVE afterwards.
Engine split per routing pass chunk:
  DVE: logits-mult, y = e*u_hat, reciprocal; ScalarE: exp;
  GpSimd: softmax denominator pair-tree + normalize-mult; PE: capsule folds.
"""

import os
import sys

import numpy as np

sys.path.insert(0, "/opt/trn_rl_repo")

B, IC, ID = 256, 1152, 8
NC, DC = 10, 16
NCORES = 8
BC = B // NCORES            # 32 batches per core
SB = 2                      # sub-batches per core
BB = BC // SB               # 16 batches per sub-batch
IPK = 8                     # input capsules packed per group
G2 = IC // IPK              # 144 groups
K2 = IPK * ID               # 64 contraction rows
ND = NC * DC                # 160
FREE2 = G2 * ND             # 23040
CH = 12                     # chunk size in groups (production & routing)
NCH = G2 // CH              # 12 chunks
PSUM_GRP = 3                # groups per PSUM bank tile (3*160*4B < 2KB)
EPS = 1e-7

_CACHE = {}


def _build_nc(reps=1):
    # reps > 1 wraps the whole computation in an on-device loop; used only by
    # the timing harness (delta of two builds cancels dispatch/transfer cost).
    import contextlib

    import concourse.bacc as bacc
    import concourse.mybir as mybir
    import concourse.tile as tile

    F32 = mybir.dt.float32
    F32R = mybir.dt.float32r
    ALU = mybir.AluOpType
    ACTF = mybir.ActivationFunctionType

    nc = bacc.Bacc()
    # lt: sub-batch 2's block-diagonal stationary tiles (DMA'd under the
    # routing overlap). Sub-batch 1 ships compact (xt) and is expanded
    # on-device (GpSimd mask-multiply) to cut the serial P1 DMA.
    lt_d = nc.dram_tensor("lt", [K2, G2 * 128], F32, kind="ExternalInput")
    xt_d = nc.dram_tensor("xt", [K2, G2 * BB], F32, kind="ExternalInput")
    mlt_d = nc.dram_tensor("mlt", [K2, 128], F32, kind="ExternalInput")
    wr_d = nc.dram_tensor("wr", [K2, FREE2], F32, kind="ExternalInput")
    mask_d = nc.dram_tensor("mask", [128, BB], F32, kind="ExternalInput")
    out_d = nc.dram_tensor("out", [BC, ND], F32, kind="ExternalOutput")

    with tile.TileContext(nc) as tc:
        with (
            tc.tile_pool(name="const", bufs=1) as cpool,
            tc.tile_pool(name="sq", bufs=1) as qpool,
            tc.tile_pool(name="uhp", bufs=NCH) as uhp,
            tc.tile_pool(name="psw", bufs=2, space="PSUM") as swpool,
            tc.tile_pool(name="ltp", bufs=3) as ltp,
            tc.tile_pool(name="wrp", bufs=3) as wrp,
            tc.tile_pool(name="pprod", bufs=6, space="PSUM") as pprod,
            tc.tile_pool(name="x", bufs=3) as xpool,
            tc.tile_pool(name="y", bufs=3) as ypool,
            tc.tile_pool(name="dn", bufs=2) as dnpool,
            tc.tile_pool(name="s1t", bufs=1) as s1pool,
            tc.tile_pool(name="rv", bufs=2) as rvpool,
        ):
            mask_t = cpool.tile([128, BB], F32)
            nc.sync.dma_start(mask_t[:], mask_d[:])
            mlt_t = cpool.tile([K2, 128], F32)
            nc.sync.dma_start(mlt_t[:], mlt_d[:])
            mask_r = cpool.tile([128, BB], F32R)
            nc.vector.tensor_copy(mask_r[:], mask_t[:])
            # V is kept replicated 8x across partitions (p%16 = b) so the
            # squash chain directly produces the broadcast tile for the next
            # pass's logits; only one small broadcast DMA per transition.
            V = cpool.tile([128, ND], F32)

            rep_ctx = (
                tc.For_i(0, reps, 1) if reps > 1 else contextlib.nullcontext()
            )

            def collapse3(ps_w, sc):
                # s_sb = (blk0+blk1+blk2) of [16, 480] PSUM -> [16, 160] SBUF
                cw = qpool.tile([BB, PSUM_GRP * ND], F32, tag="c3_w")
                nc.scalar.activation(cw[:], ps_w[:], ACTF.Copy, scale=sc)
                s3 = qpool.tile([BB, ND], F32, tag="c3_a")
                nc.vector.tensor_add(
                    s3[:], cw[:, 0:ND], cw[:, ND:2 * ND]
                )
                s = qpool.tile([BB, ND], F32, tag="c3_s")
                nc.vector.tensor_add(s[:], s3[:], cw[:, 2 * ND:3 * ND])
                return s

            def bcast16(s):
                # replicate [16, ND] -> [128, ND] (8 partition quadrants)
                s128 = qpool.tile([128, ND], F32, tag="s128")
                for q in range(IPK):
                    nc.sync.dma_start(s128[q * BB:(q + 1) * BB, :], s[:, :])
                return s128

            def squash(s, vt, P=128):
                # vt = squash(s); [P, ND] f32, tiny
                sq = qpool.tile([P, ND], F32, tag="sq_sq")
                nc.vector.tensor_mul(sq[:], s[:], s[:])
                se = qpool.tile([P, ND], F32, tag="sq_se")
                nc.vector.tensor_scalar_add(se[:], sq[:], EPS)
                a = qpool.tile([P, ND], F32, tag="sq_a")
                nc.scalar.activation(a[:], se[:], ACTF.Sqrt)
                d2 = qpool.tile([P, ND], F32, tag="sq_d2")
                nc.vector.scalar_tensor_tensor(
                    d2[:], sq[:], 1.0, a[:], op0=ALU.add, op1=ALU.mult
                )
                r = qpool.tile([P, ND], F32, tag="sq_r")
                nc.vector.reciprocal(r[:], d2[:])
                t1 = qpool.tile([P, ND], F32, tag="sq_t1")
                nc.vector.tensor_mul(t1[:], s[:], sq[:])
                nc.vector.tensor_mul(vt[:], t1[:], r[:])

            with rep_ctx:
              for s_i in range(SB):
                # ---------- production: u_hat + s1 fold ----------
                uch = []
                s1parts = []
                for c in range(NCH):
                    g0 = c * CH
                    ltt = ltp.tile([K2, CH * 128], F32, tag="ltt")
                    if s_i == 0:
                        xtt = ltp.tile([K2, CH * BB], F32, tag="xtt")
                        nc.sync.dma_start(
                            xtt[:], xt_d[:, g0 * BB:(g0 + CH) * BB]
                        )
                        ltt4 = ltt[:].rearrange(
                            "p (g i b) -> p g i b", i=IPK, b=BB
                        )
                        xt_b = (
                            xtt[:]
                            .rearrange("p (g b) -> p g b", b=BB)
                            .unsqueeze(2)
                            .broadcast_to([K2, CH, IPK, BB])
                        )
                        ml_b = (
                            mlt_t[:]
                            .rearrange("p (i b) -> p i b", b=BB)
                            .unsqueeze(1)
                            .broadcast_to([K2, CH, IPK, BB])
                        )
                        nc.gpsimd.tensor_tensor(ltt4, xt_b, ml_b, ALU.mult)
                    else:
                        nc.sync.dma_start(
                            ltt[:],
                            lt_d[:, g0 * 128:(g0 + CH) * 128],
                        )
                    wrt = wrp.tile([K2, CH * ND], F32)
                    nc.sync.dma_start(
                        wrt[:], wr_d[:, g0 * ND:(g0 + CH) * ND]
                    )
                    u = uhp.tile([128, CH * ND], F32, tag="uh")
                    uch.append(u)
                    for t3 in range(CH // PSUM_GRP):
                        pt = pprod.tile([128, PSUM_GRP * ND], F32)
                        for j in range(PSUM_GRP):
                            gl = t3 * PSUM_GRP + j
                            nc.tensor.matmul(
                                pt[:, j * ND:(j + 1) * ND],
                                ltt[:, gl * 128:(gl + 1) * 128],
                                wrt[:, gl * ND:(gl + 1) * ND],
                                start=True,
                                stop=True,
                            )
                        lo = t3 * PSUM_GRP * ND
                        hi = (t3 + 1) * PSUM_GRP * ND
                        nc.scalar.copy(u[:, lo:hi], pt[:])
                    # s1 partial: sum over the chunk's 12 groups on DVE
                    u3c = u[:].rearrange("p (g nd) -> p g nd", nd=ND)
                    t6 = s1pool.tile([128, 6 * ND], F32, tag="s1a")
                    t63 = t6[:].rearrange("p (g nd) -> p g nd", nd=ND)
                    nc.vector.tensor_tensor(
                        t63, u3c[:, 0:12:2, :], u3c[:, 1:12:2, :], ALU.add
                    )
                    t3_ = s1pool.tile([128, 3 * ND], F32, tag="s1b")
                    t33 = t3_[:].rearrange("p (g nd) -> p g nd", nd=ND)
                    nc.vector.tensor_tensor(
                        t33, t63[:, 0:6:2, :], t63[:, 1:6:2, :], ALU.add
                    )
                    sp = s1pool.tile([128, ND], F32, tag="s1p")
                    nc.vector.tensor_add(sp[:], t33[:, 0, :], t33[:, 1, :])
                    nc.vector.tensor_add(sp[:], sp[:], t33[:, 2, :])
                    if c == 0:
                        s1acc = cpool.tile([128, ND], F32, tag="s1acc")
                        nc.vector.tensor_copy(s1acc[:], sp[:])
                    else:
                        nc.vector.tensor_add(s1acc[:], s1acc[:], sp[:])
                # partition fold (i8 quadrants -> b) via one plain-f32 matmul
                ps1 = swpool.tile([BB, ND], F32, tag="psw")
                nc.tensor.matmul(ps1[:], mask_t[:], s1acc[:],
                                 start=True, stop=True)
                s1 = qpool.tile([BB, ND], F32, tag="c3_s")
                nc.scalar.activation(s1[:], ps1[:], ACTF.Copy, scale=0.1)
                squash(bcast16(s1), V)

                # ---------- routing passes 2 and 3 ----------
                for t in (2, 3):
                    ps_w = swpool.tile([BB, PSUM_GRP * ND], F32, tag="psw")
                    n_fold = 0
                    for c in range(NCH):
                        u = uch[c]
                        x = xpool.tile([128, CH * ND], F32)
                        x3 = x[:].rearrange("p (g nd) -> p g nd", nd=ND)
                        u3 = u[:].rearrange("p (g nd) -> p g nd", nd=ND)
                        vb_b = V[:].unsqueeze(1).broadcast_to(
                            [128, CH, ND]
                        )
                        # logits = u_hat * V (broadcast over groups)
                        nc.vector.tensor_tensor(x3, u3, vb_b, ALU.mult)
                        nc.scalar.activation(x[:], x[:], ACTF.Exp)
                        # y = e * u_hat, parallel with the GP denom tree;
                        # alternate DVE/GP per chunk for engine balance.
                        # pass 2 folds in exact f32 (its error feeds back
                        # through V and is chaotically amplified); pass 3
                        # folds in f32r (error hits the output directly).
                        y = ypool.tile([128, CH * ND],
                                       F32 if t == 2 else F32R,
                                       tag="y")
                        y_eng = nc.vector if c % 2 == 0 else nc.gpsimd
                        y_eng.tensor_tensor(
                            y[:], x[:], u[:], ALU.mult
                        )
                        # denominator pair-tree over n on GpSimd
                        x4 = x[:].rearrange(
                            "p (g n d) -> p g n d", n=NC, d=DC
                        )
                        d5 = dnpool.tile([128, CH * 5 * DC], F32, tag="d5")
                        d53 = d5[:].rearrange(
                            "p (g n d) -> p g n d", n=5, d=DC
                        )
                        nc.gpsimd.tensor_tensor(
                            d53, x4[:, :, 0:10:2, :], x4[:, :, 1:10:2, :],
                            ALU.add,
                        )
                        dp = dnpool.tile([128, CH * 2 * DC], F32, tag="dp")
                        dp3 = dp[:].rearrange(
                            "p (g n d) -> p g n d", n=2, d=DC
                        )
                        nc.gpsimd.tensor_tensor(
                            dp3, d53[:, :, 0:4:2, :], d53[:, :, 1:4:2, :],
                            ALU.add,
                        )
                        dn = dnpool.tile([128, CH * DC], F32, tag="dn")
                        dn3 = dn[:].rearrange("p (g d) -> p g d", d=DC)
                        nc.gpsimd.tensor_tensor(
                            dn3, dp3[:, :, 0, :], dp3[:, :, 1, :], ALU.add
                        )
                        nc.gpsimd.tensor_tensor(
                            dn3, dn3, d53[:, :, 4, :], ALU.add
                        )
                        rv = rvpool.tile([128, CH * DC], F32)
                        nc.vector.reciprocal(rv[:], dn[:])
                        rv_b = (
                            rv[:]
                            .rearrange("p (g d) -> p g d", d=DC)
                            .unsqueeze(2)
                            .broadcast_to([128, CH, NC, DC])
                        )
                        y4 = y[:].rearrange(
                            "p (g n d) -> p g n d", n=NC, d=DC
                        )
                        nc.gpsimd.tensor_tensor(y4, y4, rv_b, ALU.mult)
                        mk = mask_t if t == 2 else mask_r
                        for j3 in range(CH // PSUM_GRP):
                            nc.tensor.matmul(
                                ps_w[:],
                                mk[:],
                                y[:, j3 * PSUM_GRP * ND:
                                  (j3 + 1) * PSUM_GRP * ND],
                                start=(n_fold == 0),
                                stop=(n_fold == G2 // PSUM_GRP - 1),
                            )
                            n_fold += 1
                    s_t = collapse3(ps_w, 1.0)
                    vt = qpool.tile([128, ND], F32, tag="vt")
                    squash(bcast16(s_t), vt)
                    if t == 2:
                        nc.vector.tensor_add(V[:], V[:], vt[:])
                    else:
                        nc.sync.dma_start(
                            out_d[s_i * BB:(s_i + 1) * BB, :],
                            vt[0:BB, :],
                        )
    nc.finalize()
    return nc


def _host_pack(inputs, W):
    """Build per-core LT, shared WR and mask, all f32."""
    inputs = np.ascontiguousarray(inputs, dtype=np.float32)
    W = np.ascontiguousarray(W, dtype=np.float32)

    # WR[r=(i8*8+k), g*160 + n*16 + d] = W[n, g*8+i8, d, k]
    W6 = W.reshape(NC, G2, IPK, DC, ID)
    wr = np.ascontiguousarray(
        W6.transpose(2, 4, 1, 0, 3).reshape(K2, FREE2)
    )

    mask = np.ascontiguousarray(
        np.tile(np.eye(BB, dtype=np.float32), (IPK, 1))
    )
    # mlt[r=(i8*8+k), i8'*16+b] = (i8 == i8')
    mlt = np.zeros((K2, 128), dtype=np.float32)
    for i8 in range(IPK):
        mlt[i8 * ID:(i8 + 1) * ID, i8 * BB:(i8 + 1) * BB] = 1.0

    lts, xts = [], []
    for core in range(NCORES):
        xc = inputs[core * BC:(core + 1) * BC]              # [BC, IC, ID]
        x6 = xc.reshape(SB, BB, G2, IPK, ID)                # [s, b, g, i8, k]
        # sub-batch 2: full block-diagonal layout
        lt = np.zeros((K2, G2, 128), dtype=np.float32)
        for i8 in range(IPK):
            lt[i8 * ID:(i8 + 1) * ID, :, i8 * BB:(i8 + 1) * BB] = (
                x6[1, :, :, i8, :].transpose(2, 1, 0)       # [k, g, b]
            )
        lts.append(np.ascontiguousarray(lt.reshape(K2, G2 * 128)))
        # sub-batch 1: compact transposed inputs
        xt = np.zeros((K2, G2, BB), dtype=np.float32)
        for i8 in range(IPK):
            xt[i8 * ID:(i8 + 1) * ID] = x6[0, :, :, i8, :].transpose(2, 1, 0)
        xts.append(np.ascontiguousarray(xt.reshape(K2, G2 * BB)))
    return lts, xts, wr, mask, mlt


def kernel(inputs, W):
    from concourse.bass_utils import run_bass_kernel_spmd

    if "nc" not in _CACHE:
        _CACHE["nc"] = _build_nc()
    nc = _CACHE["nc"]

    lts, xts, wr, mask, mlt = _host_pack(np.asarray(inputs), np.asarray(W))
    in_maps = [
        {"lt": lts[c], "xt": xts[c], "wr": wr, "mask": mask, "mlt": mlt}
        for c in range(NCORES)
    ]
    res = run_bass_kernel_spmd(nc, in_maps, core_ids=list(range(NCORES)))
    outs = [
        np.asarray(res.results[c]["out"]).reshape(BC, NC, DC, 1)
        for c in range(NCORES)
    ]
    return np.concatenate(outs, axis=0).astype(np.float32)


if __name__ == "__main__":
    rng = np.random.default_rng(0)
    x = rng.standard_normal((B, IC, ID), dtype=np.float32)
    w = rng.standard_normal((NC, IC, DC, ID), dtype=np.float32) * 0.1
    out = kernel(x, w)
    print(out.shape, out.dtype)


# revision 29
# speedup vs baseline: 9.8390x; 1.2020x over previous
"""CapsuleLayer (dynamic routing) Trainium2 Bass kernel.

Full-input contract: kernel(inputs, W) -> [256, 10, 16, 1] f32.
Data-parallel over batch: 8 cores x 32 batches, W replicated.

Math restructuring vs the reference:
  - routing logits are always b_t = u_hat * V_t with V_t = sum of previous
    squashed outputs (broadcast over IC), so no [B,NC,IC,DC] logits tensor is
    ever materialized; only the running V[b,n,d] is kept.
  - pass 1 (uniform softmax) reduces to s1 = 0.1 * sum_i u_hat, accumulated
    on the DVE during u_hat production (idle there) + one PE partition-fold.

Everything is kept in f32: the routing iteration chaotically amplifies
perturbations (~700x), so bf16/f16 intermediates destroy accuracy. The only
reduced-precision concession is float32r (~12-bit mantissa) on the PASS-3
capsule-fold matmuls, whose rounding hits the output directly without
amplification; pass-2 folds stay plain f32 (their error feeds back through V
and grows ~20-40x). Measured: 4.8e-4 absmax vs the f32 reference.

Per core the 32 local batches are processed as 2 serial sub-batches of 16 so
the f32 u_hat stays SBUF-resident (no DRAM spill):
  partition p = i8*16 + b   (8 input capsules packed per "group", 144 groups)
  u_hat: 12 chunk tiles [128, 12*160] in a 12-slot pool; sub-batch 2's
    production reuses sub-batch 1's slots chunk-by-chunk, so its DMA/PE work
    overlaps sub-batch 1's final routing pass.
  production: u[p, g*160+nd] = sum_{r=(i8,k)} LT[r, g*128+p] * WR[r, g*160+nd]
    with LT the block-diagonal input transposes (rows r = i8*8+k, 16-col
    diagonal blocks) and WR the matching W slices - both prepacked on host.
  capsule fold: s[b, nd] = sum_p mask[p, b] * w[p, nd] via PE (mask = eye(16)
    tiled 8x), accumulated 3 groups wide ([16, 480] PSUM) to amortize weight
    loads; the 3 column blocks are summed on DVE afterwards.
Engine split per routing pass chunk (all elementwise on DVE - measured
faster on HW than offloading to GpSimd, whose software loops handle the
strided/broadcast access patterns poorly):
  DVE: logits-mult, softmax-denominator reduce, reciprocal, normalize-mult,
  weight-mult; ScalarE: exp (+ PSUM->SBUF copies); PE: matmuls/capsule folds.
"""

import os
import sys

import numpy as np

sys.path.insert(0, "/opt/trn_rl_repo")

B, IC, ID = 256, 1152, 8
NC, DC = 10, 16
NCORES = 8
BC = B // NCORES            # 32 batches per core
SB = 2                      # sub-batches per core
BB = BC // SB               # 16 batches per sub-batch
IPK = 8                     # input capsules packed per group
G2 = IC // IPK              # 144 groups
K2 = IPK * ID               # 64 contraction rows
ND = NC * DC                # 160
FREE2 = G2 * ND             # 23040
CH = 12                     # chunk size in groups (production & routing)
NCH = G2 // CH              # 12 chunks
PSUM_GRP = 3                # groups per PSUM bank tile (3*160*4B < 2KB)
EPS = 1e-7

_CACHE = {}


def _build_nc(reps=1, skip_routing=False, gp_mode="dve"):
    # reps > 1 wraps the whole computation in an on-device loop; used only by
    # the timing harness (delta of two builds cancels dispatch/transfer cost).
    import contextlib

    import concourse.bacc as bacc
    import concourse.mybir as mybir
    import concourse.tile as tile

    F32 = mybir.dt.float32
    F32R = mybir.dt.float32r
    ALU = mybir.AluOpType
    ACTF = mybir.ActivationFunctionType

    nc = bacc.Bacc()
    # lt: sub-batch 2's block-diagonal stationary tiles (DMA'd under the
    # routing overlap). Sub-batch 1 ships compact (xt) and is expanded
    # on-device (GpSimd mask-multiply) to cut the serial P1 DMA.
    lt_d = nc.dram_tensor("lt", [K2, G2 * 128], F32, kind="ExternalInput")
    xt_d = nc.dram_tensor("xt", [K2, G2 * BB], F32, kind="ExternalInput")
    mlt_d = nc.dram_tensor("mlt", [K2, 128], F32, kind="ExternalInput")
    wr_d = nc.dram_tensor("wr", [K2, FREE2], F32, kind="ExternalInput")
    mask_d = nc.dram_tensor("mask", [128, BB], F32, kind="ExternalInput")
    out_d = nc.dram_tensor("out", [BC, ND], F32, kind="ExternalOutput")

    with tile.TileContext(nc) as tc:
        with (
            tc.tile_pool(name="const", bufs=1) as cpool,
            tc.tile_pool(name="sq", bufs=1) as qpool,
            tc.tile_pool(name="uhp", bufs=NCH) as uhp,
            tc.tile_pool(name="psw", bufs=2, space="PSUM") as swpool,
            tc.tile_pool(name="ltp", bufs=3) as ltp,
            tc.tile_pool(name="wrp", bufs=3) as wrp,
            tc.tile_pool(name="pprod", bufs=6, space="PSUM") as pprod,
            tc.tile_pool(name="x", bufs=3) as xpool,
            tc.tile_pool(name="y", bufs=3) as ypool,
            tc.tile_pool(name="dn", bufs=2) as dnpool,
            tc.tile_pool(name="s1t", bufs=1) as s1pool,
            tc.tile_pool(name="rv", bufs=2) as rvpool,
        ):
            mask_t = cpool.tile([128, BB], F32)
            nc.sync.dma_start(mask_t[:], mask_d[:])
            mlt_t = cpool.tile([K2, 128], F32)
            nc.sync.dma_start(mlt_t[:], mlt_d[:])
            mask_r = cpool.tile([128, BB], F32R)
            nc.vector.tensor_copy(mask_r[:], mask_t[:])
            # V is kept replicated 8x across partitions (p%16 = b) so the
            # squash chain directly produces the broadcast tile for the next
            # pass's logits; only one small broadcast DMA per transition.
            V = cpool.tile([128, ND], F32)

            rep_ctx = (
                tc.For_i(0, reps, 1) if reps > 1 else contextlib.nullcontext()
            )

            def collapse3(ps_w, sc):
                # s_sb = (blk0+blk1+blk2) of [16, 480] PSUM -> [16, 160] SBUF
                cw = qpool.tile([BB, PSUM_GRP * ND], F32, tag="c3_w")
                nc.scalar.activation(cw[:], ps_w[:], ACTF.Copy, scale=sc)
                s3 = qpool.tile([BB, ND], F32, tag="c3_a")
                nc.vector.tensor_add(
                    s3[:], cw[:, 0:ND], cw[:, ND:2 * ND]
                )
                s = qpool.tile([BB, ND], F32, tag="c3_s")
                nc.vector.tensor_add(s[:], s3[:], cw[:, 2 * ND:3 * ND])
                return s

            def bcast16(s):
                # replicate [16, ND] -> [128, ND] (8 partition quadrants)
                s128 = qpool.tile([128, ND], F32, tag="s128")
                for q in range(IPK):
                    nc.sync.dma_start(s128[q * BB:(q + 1) * BB, :], s[:, :])
                return s128

            def squash(s, vt, P=128):
                # vt = squash(s); [P, ND] f32, tiny
                sq = qpool.tile([P, ND], F32, tag="sq_sq")
                nc.vector.tensor_mul(sq[:], s[:], s[:])
                se = qpool.tile([P, ND], F32, tag="sq_se")
                nc.vector.tensor_scalar_add(se[:], sq[:], EPS)
                a = qpool.tile([P, ND], F32, tag="sq_a")
                nc.scalar.activation(a[:], se[:], ACTF.Sqrt)
                d2 = qpool.tile([P, ND], F32, tag="sq_d2")
                nc.vector.scalar_tensor_tensor(
                    d2[:], sq[:], 1.0, a[:], op0=ALU.add, op1=ALU.mult
                )
                r = qpool.tile([P, ND], F32, tag="sq_r")
                nc.vector.reciprocal(r[:], d2[:])
                t1 = qpool.tile([P, ND], F32, tag="sq_t1")
                nc.vector.tensor_mul(t1[:], s[:], sq[:])
                nc.vector.tensor_mul(vt[:], t1[:], r[:])

            with rep_ctx:
              for s_i in range(SB):
                # ---------- production: u_hat + s1 fold ----------
                uch = []
                s1parts = []
                for c in range(NCH):
                    g0 = c * CH
                    ltt = ltp.tile([K2, CH * 128], F32, tag="ltt")
                    if s_i == 0:
                        xtt = ltp.tile([K2, CH * BB], F32, tag="xtt")
                        nc.sync.dma_start(
                            xtt[:], xt_d[:, g0 * BB:(g0 + CH) * BB]
                        )
                        ltt4 = ltt[:].rearrange(
                            "p (g i b) -> p g i b", i=IPK, b=BB
                        )
                        xt_b = (
                            xtt[:]
                            .rearrange("p (g b) -> p g b", b=BB)
                            .unsqueeze(2)
                            .broadcast_to([K2, CH, IPK, BB])
                        )
                        ml_b = (
                            mlt_t[:]
                            .rearrange("p (i b) -> p i b", b=BB)
                            .unsqueeze(1)
                            .broadcast_to([K2, CH, IPK, BB])
                        )
                        nc.gpsimd.tensor_tensor(ltt4, xt_b, ml_b, ALU.mult)
                    else:
                        nc.sync.dma_start(
                            ltt[:],
                            lt_d[:, g0 * 128:(g0 + CH) * 128],
                        )
                    wrt = wrp.tile([K2, CH * ND], F32)
                    nc.sync.dma_start(
                        wrt[:], wr_d[:, g0 * ND:(g0 + CH) * ND]
                    )
                    u = uhp.tile([128, CH * ND], F32, tag="uh")
                    uch.append(u)
                    for t3 in range(CH // PSUM_GRP):
                        pt = pprod.tile([128, PSUM_GRP * ND], F32)
                        for j in range(PSUM_GRP):
                            gl = t3 * PSUM_GRP + j
                            nc.tensor.matmul(
                                pt[:, j * ND:(j + 1) * ND],
                                ltt[:, gl * 128:(gl + 1) * 128],
                                wrt[:, gl * ND:(gl + 1) * ND],
                                start=True,
                                stop=True,
                            )
                        lo = t3 * PSUM_GRP * ND
                        hi = (t3 + 1) * PSUM_GRP * ND
                        nc.scalar.copy(u[:, lo:hi], pt[:])
                    # s1 partial: sum over the chunk's 12 groups on DVE
                    u3c = u[:].rearrange("p (g nd) -> p g nd", nd=ND)
                    t6 = s1pool.tile([128, 6 * ND], F32, tag="s1a")
                    t63 = t6[:].rearrange("p (g nd) -> p g nd", nd=ND)
                    nc.vector.tensor_tensor(
                        t63, u3c[:, 0:12:2, :], u3c[:, 1:12:2, :], ALU.add
                    )
                    t3_ = s1pool.tile([128, 3 * ND], F32, tag="s1b")
                    t33 = t3_[:].rearrange("p (g nd) -> p g nd", nd=ND)
                    nc.vector.tensor_tensor(
                        t33, t63[:, 0:6:2, :], t63[:, 1:6:2, :], ALU.add
                    )
                    sp = s1pool.tile([128, ND], F32, tag="s1p")
                    nc.vector.tensor_add(sp[:], t33[:, 0, :], t33[:, 1, :])
                    nc.vector.tensor_add(sp[:], sp[:], t33[:, 2, :])
                    if c == 0:
                        s1acc = cpool.tile([128, ND], F32, tag="s1acc")
                        nc.vector.tensor_copy(s1acc[:], sp[:])
                    else:
                        nc.vector.tensor_add(s1acc[:], s1acc[:], sp[:])
                # partition fold (i8 quadrants -> b) via one plain-f32 matmul
                ps1 = swpool.tile([BB, ND], F32, tag="psw")
                nc.tensor.matmul(ps1[:], mask_t[:], s1acc[:],
                                 start=True, stop=True)
                s1 = qpool.tile([BB, ND], F32, tag="c3_s")
                nc.scalar.activation(s1[:], ps1[:], ACTF.Copy, scale=0.1)
                squash(bcast16(s1), V)

                # ---------- routing passes 2 and 3 ----------
                for t in () if skip_routing else (2, 3):
                    ps_w = swpool.tile([BB, PSUM_GRP * ND], F32, tag="psw")
                    n_fold = 0
                    for c in range(NCH):
                        u = uch[c]
                        x = xpool.tile([128, CH * ND], F32)
                        x3 = x[:].rearrange("p (g nd) -> p g nd", nd=ND)
                        u3 = u[:].rearrange("p (g nd) -> p g nd", nd=ND)
                        vb_b = V[:].unsqueeze(1).broadcast_to(
                            [128, CH, ND]
                        )
                        # logits = u_hat * V (broadcast over groups)
                        nc.vector.tensor_tensor(x3, u3, vb_b, ALU.mult)
                        nc.scalar.activation(x[:], x[:], ACTF.Exp)
                        # y = e * u_hat, parallel with the GP denom tree;
                        # alternate DVE/GP per chunk for engine balance.
                        # pass 2 folds in exact f32 (its error feeds back
                        # through V and is chaotically amplified); pass 3
                        # folds in f32r (error hits the output directly).
                        y = ypool.tile([128, CH * ND],
                                       F32 if t == 2 else F32R,
                                       tag="y")
                        y_eng = (nc.vector if (c % 2 == 0 or gp_mode == "dve")
                                 else nc.gpsimd)
                        y_eng.tensor_tensor(
                            y[:], x[:], u[:], ALU.mult
                        )
                        # denominator pair-tree over n on GpSimd
                        x4 = x[:].rearrange(
                            "p (g n d) -> p g n d", n=NC, d=DC
                        )
                        # denom = sum over n: one strided reduce on DVE
                        dn = dnpool.tile([128, CH * DC], F32, tag="dn")
                        dn4 = dn[:].rearrange(
                            "p (g o d) -> p g o d", o=1, d=DC
                        )
                        nc.vector.tensor_reduce(
                            dn4,
                            x4.transpose([0, 1, 3, 2]),
                            axis=mybir.AxisListType.X,
                            op=ALU.add,
                        )
                        rv = rvpool.tile([128, CH * DC], F32)
                        nc.vector.reciprocal(rv[:], dn[:])
                        rv_b = (
                            rv[:]
                            .rearrange("p (g d) -> p g d", d=DC)
                            .unsqueeze(2)
                            .broadcast_to([128, CH, NC, DC])
                        )
                        y4 = y[:].rearrange(
                            "p (g n d) -> p g n d", n=NC, d=DC
                        )
                        nc.vector.tensor_tensor(y4, y4, rv_b, ALU.mult)
                        mk = mask_t if t == 2 else mask_r
                        for j3 in range(CH // PSUM_GRP):
                            nc.tensor.matmul(
                                ps_w[:],
                                mk[:],
                                y[:, j3 * PSUM_GRP * ND:
                                  (j3 + 1) * PSUM_GRP * ND],
                                start=(n_fold == 0),
                                stop=(n_fold == G2 // PSUM_GRP - 1),
                            )
                            n_fold += 1
                    s_t = collapse3(ps_w, 1.0)
                    vt = qpool.tile([128, ND], F32, tag="vt")
                    squash(bcast16(s_t), vt)
                    if t == 2:
                        nc.vector.tensor_add(V[:], V[:], vt[:])
                    else:
                        nc.sync.dma_start(
                            out_d[s_i * BB:(s_i + 1) * BB, :],
                            vt[0:BB, :],
                        )
            if skip_routing:
                for s_i in range(SB):
                    nc.sync.dma_start(
                        out_d[s_i * BB:(s_i + 1) * BB, :], V[0:BB, :]
                    )
    nc.finalize()
    return nc


def _host_pack(inputs, W):
    """Build per-core LT, shared WR and mask, all f32."""
    inputs = np.ascontiguousarray(inputs, dtype=np.float32)
    W = np.ascontiguousarray(W, dtype=np.float32)

    # WR[r=(i8*8+k), g*160 + n*16 + d] = W[n, g*8+i8, d, k]
    W6 = W.reshape(NC, G2, IPK, DC, ID)
    wr = np.ascontiguousarray(
        W6.transpose(2, 4, 1, 0, 3).reshape(K2, FREE2)
    )

    mask = np.ascontiguousarray(
        np.tile(np.eye(BB, dtype=np.float32), (IPK, 1))
    )
    # mlt[r=(i8*8+k), i8'*16+b] = (i8 == i8')
    mlt = np.zeros((K2, 128), dtype=np.float32)
    for i8 in range(IPK):
        mlt[i8 * ID:(i8 + 1) * ID, i8 * BB:(i8 + 1) * BB] = 1.0

    lts, xts = [], []
    for core in range(NCORES):
        xc = inputs[core * BC:(core + 1) * BC]              # [BC, IC, ID]
        x6 = xc.reshape(SB, BB, G2, IPK, ID)                # [s, b, g, i8, k]
        # sub-batch 2: full block-diagonal layout
        lt = np.zeros((K2, G2, 128), dtype=np.float32)
        for i8 in range(IPK):
            lt[i8 * ID:(i8 + 1) * ID, :, i8 * BB:(i8 + 1) * BB] = (
                x6[1, :, :, i8, :].transpose(2, 1, 0)       # [k, g, b]
            )
        lts.append(np.ascontiguousarray(lt.reshape(K2, G2 * 128)))
        # sub-batch 1: compact transposed inputs
        xt = np.zeros((K2, G2, BB), dtype=np.float32)
        for i8 in range(IPK):
            xt[i8 * ID:(i8 + 1) * ID] = x6[0, :, :, i8, :].transpose(2, 1, 0)
        xts.append(np.ascontiguousarray(xt.reshape(K2, G2 * BB)))
    return lts, xts, wr, mask, mlt


def kernel(inputs, W):
    from concourse.bass_utils import run_bass_kernel_spmd

    if "nc" not in _CACHE:
        _CACHE["nc"] = _build_nc()
    nc = _CACHE["nc"]

    lts, xts, wr, mask, mlt = _host_pack(np.asarray(inputs), np.asarray(W))
    in_maps = [
        {"lt": lts[c], "xt": xts[c], "wr": wr, "mask": mask, "mlt": mlt}
        for c in range(NCORES)
    ]
    res = run_bass_kernel_spmd(nc, in_maps, core_ids=list(range(NCORES)))
    outs = [
        np.asarray(res.results[c]["out"]).reshape(BC, NC, DC, 1)
        for c in range(NCORES)
    ]
    return np.concatenate(outs, axis=0).astype(np.float32)


if __name__ == "__main__":
    rng = np.random.default_rng(0)
    x = rng.standard_normal((B, IC, ID), dtype=np.float32)
    w = rng.standard_normal((NC, IC, DC, ID), dtype=np.float32) * 0.1
    out = kernel(x, w)
    print(out.shape, out.dtype)
